# revision 52
# baseline (speedup 1.0000x reference)
"""Trainium2 Bass kernel for nn_AveragedAdapter (dense_mlp).

Computes: loss = sum_{a,e} mean_{b,d} (gelu(f[:,a] @ W1[a,e] + b1[a,e]) @ W2[a,e]
                                        + b2[a,e] - target[:,a])^2 / E

Sharding: expert-parallel over the first expert axis `a` — core a computes the
full inner-e loop for its adapter row and returns per-partition partial sums of
squared errors; the host sums the 8x128 partials and applies the 1/(B*D*E)
scale.

The 512 MiB of weights dominate the roofline (each element used exactly once),
so weights (plus features and the hidden activations) are carried in fp8-e4m3.
Biases, targets and all accumulation stay >= bf16 (matmuls accumulate in fp32
PSUM).

Per-core program (a = core id):
  - W1[a],W2[a] packed host-side into one [E, 128, 16384] fp8 slab
    (partition-major; cols 0..8192 = W1 k-chunks, 8192..16384 = W2 k-chunk
    pairs). The sync HWDGE ring is FIFO, so slabs are issued in consumption
    order: all W1 slabs, then W2 slabs, with the LAST W2 slab split in three
    pieces (6/1/1 chunk-pairs) so its matmuls chase the stream — only ~128KB
    plus one DoubleRow matmul gate the loss tail after the final byte.
    Small inputs (features, b1, shared target, b2 rows) ride the scalar
    ring; the per-expert [B,E,D] target tiles of the original design are
    gone (~1MB less HBM traffic).
  - phase 1 (all experts): layer 1 computes hT (H on partitions) with W1
    chunk-pairs stationary via fp8 DoubleRow; bias add on DVE -> bf16;
    exact-erf Gelu on ACT -> fp8 h kept in SBUF for all 8 experts.
  - phase 2 (all experts): 8 fp8 DoubleRow matmuls accumulate h @ W2 into a
    PSUM bank; start=True rides the FIRST matmul (whose natural wait is the
    W2 DMA semaphore — carrying it on any other group member serializes each
    expert behind the previous expert's DVE subtract, ~1.3us/expert); b2[e]
    is folded in as a bf16 rank-1 ones-row matmul slotted second (start
    =False), so no [B,E,D] target prep is needed. Then err = psum - target
    on DVE (bf16), Square+row-accumulate on ACT into a column of one
    [128,8] accumulator. pso bufs=4 so the start-group zeroing never waits
    on a recent reader.
  - output: one final matmul folds partitions -> [1,8] (32B, single
    descriptor; a [128,1] output would emit 128 4-byte descriptors, each an
    HBM read-modify-write — measured ~8us). Host sums 8 floats per core.
"""

import sys

if "/opt/trn_rl_repo" not in sys.path:
    sys.path.insert(0, "/opt/trn_rl_repo")

import numpy as np
import ml_dtypes

B, E, D, M = 128, 8, 512, 4
H = M * D            # 2048
P = 128
KC1 = D // P         # 4  k-chunks in layer 1
MC = H // P          # 16 m-chunks of H / k-chunks in layer 2
NG = 4               # m-chunk groups (4 chunks -> one PSUM bank)
W1_COLS = KC1 * H    # 8192
W2_COLS = MC * D     # 8192
F8 = ml_dtypes.float8_e4m3
BF16 = ml_dtypes.bfloat16

_NC = None


def _build_nc(act="gelu", loss_mode="act", use_b2fold=True, split_w2=True):
    import concourse.tile as tile
    from concourse import bacc, mybir

    act_fn = {
        "gelu": mybir.ActivationFunctionType.Gelu,
        "identity": mybir.ActivationFunctionType.Identity,
    }[act]
    # Bacc (not Bass): its compile() pass legalizes sync waits for the trn2
    # ISA's one-wait-per-instruction limit.
    nc = bacc.Bacc(None)
    f8 = mybir.dt.float8e4
    f32 = mybir.dt.float32
    bf16 = mybir.dt.bfloat16

    wpack = nc.dram_tensor("wpack", [E, P, W1_COLS + W2_COLS], f8, kind="ExternalInput")
    ftp = nc.dram_tensor("ftp", [P, KC1, B], f8, kind="ExternalInput")
    b1p = nc.dram_tensor("b1p", [P, E, MC], f32, kind="ExternalInput")
    tgtp = nc.dram_tensor("tgtp", [B, D], bf16, kind="ExternalInput")   # target[:,a]
    b2pp = nc.dram_tensor("b2pp", [1, E, D], bf16, kind="ExternalInput")  # b2[a]
    # combined target+bias per inner expert (only DMA'd when use_b2fold=False)
    t2p = nc.dram_tensor("t2p", [P, E, D], bf16, kind="ExternalInput")
    # [1,E] single-partition output: a [128,1] output would emit 128
    # four-byte descriptors, each an HBM read-modify-write (sub-512B
    # transfers) — measured ~8us of post-kernel DMA time.
    loss = nc.dram_tensor("loss", [1, E], f32, kind="ExternalOutput")

    with tile.TileContext(nc) as tc:
        with (
            tc.tile_pool(name="w1pool", bufs=E) as w1pool,
            tc.tile_pool(name="w2pool", bufs=E + 2) as w2pool,
            tc.tile_pool(name="cpool", bufs=1) as cpool,
            tc.tile_pool(name="zpool", bufs=8) as zpool,
            tc.tile_pool(name="hpool", bufs=E) as hpool,
            tc.tile_pool(name="spool", bufs=2) as spool,
            tc.tile_pool(name="rpool", bufs=E) as rpool,
            tc.tile_pool(name="psz", bufs=4, space="PSUM") as psz,
            tc.tile_pool(name="pso", bufs=4, space="PSUM") as pso,
        ):
            # Small inputs on the scalar (ACT HWDGE) ring; weight slabs own the
            # sync ring end to end.
            ft = cpool.tile([P, KC1, B], f8)
            nc.scalar.dma_start(ft[:], ftp[:])
            b1s = cpool.tile([P, E, MC], f32)
            nc.scalar.dma_start(b1s[:], b1p[:])
            tgt = cpool.tile([B, D], bf16)
            nc.scalar.dma_start(tgt[:], tgtp[:])
            b2s = cpool.tile([1, E, D], bf16)
            nc.scalar.dma_start(b2s[:], b2pp[:])
            if not use_b2fold:
                tgt2 = cpool.tile([P, E, D], bf16)
                nc.scalar.dma_start(tgt2[:], t2p[:])
            ones1 = cpool.tile([1, P], bf16)
            nc.vector.memset(ones1[:], 1.0)
            # Advance the DVE vector clock past the b1s DMA with a one-element
            # read so the bias-add TTs only need their PE wait.
            dummy = cpool.tile([1, 2], f32)
            nc.vector.tensor_copy(dummy[:, 0:1], b1s[:1, 0, :1])
            # Trigger the ACT function-set table loads NOW (first use drives
            # the PSEUDO_LOAD_ACT_FUNC_SET) so their Q_XIV DMA packets drain
            # before the weight stream floods the rings — otherwise DMA
            # engine 0 carries them mid-stream and straggles ~2.5us behind
            # the other 15 engines, delaying the last W2 bytes by as much.
            dact = cpool.tile([1, 2], f32)
            nc.vector.memset(dact[:], 0.0)
            dact2 = cpool.tile([1, 2], f32)
            nc.scalar.activation(dact2[:], dact[:], act_fn)
            nc.scalar.activation(
                dact2[:], dact[:], mybir.ActivationFunctionType.Square
            )

            # Warm the PE HAM clock-gate (idle PE runs at 1.2 GHz; sustained
            # activity unlocks 2.4 GHz) while the first weight slab is in
            # flight. 8 matmuls ~= the first slab's flight time; real L1 work
            # continues the activity streak afterwards.
            onesc = cpool.tile([P, 1], f32)
            nc.vector.memset(onesc[:], 1.0)
            wsrc = cpool.tile([P, D], f8)
            nc.vector.memset(wsrc[:], 0.0)
            pwarm = psz.tile([P, D], mybir.dt.float32, tag="zp")
            NWARM = 15
            for i in range(NWARM):
                nc.tensor.matmul(
                    pwarm[:], lhsT=wsrc[:, :P], rhs=wsrc[:],
                    start=(i == 0), stop=(i == NWARM - 1),
                )

            # Weight slab delivery order == consumption order (FIFO ring).
            w1ts, w2ts = {}, {}

            for e in range(E):
                w1ts[e] = w1pool.tile([P, W1_COLS], f8, tag="w1", name=f"w1t{e}")
                nc.sync.dma_start(w1ts[e][:], wpack[e][:, :W1_COLS])
            w2view = {
                e: wpack[e][:, W1_COLS:].rearrange(
                    "p (k two d) -> p k two d", two=2, d=D
                )
                for e in range(E)
            }
            nfull = E - 1 if split_w2 else E
            for e in range(nfull):
                w2ts[e] = w2pool.tile([P, MC // 2, 2, D], f8, tag="w2", name=f"w2t{e}")
                nc.sync.dma_start(w2ts[e][:], w2view[e])
            if split_w2:
                # Last expert's W2 in three pieces (6/1/1 chunk-pairs) so only
                # the final 128KB gates the last matmul.
                w2l = [
                    w2pool.tile([P, 6, 2, D], f8, tag="w2", name="w2t7a"),
                    w2pool.tile([P, 1, 2, D], f8, tag="w2", name="w2t7b"),
                    w2pool.tile([P, 1, 2, D], f8, tag="w2", name="w2t7c"),
                ]
                nc.sync.dma_start(w2l[0][:], w2view[E - 1][:, 0:6])
                nc.sync.dma_start(w2l[1][:], w2view[E - 1][:, 6:7])
                nc.sync.dma_start(w2l[2][:], w2view[E - 1][:, 7:8])

            # Phase 1: layer-1 + gelu for ALL experts (PE executes its queue in
            # program order; keeping layer-2 work out of this stretch lets the
            # last expert's bias/gelu chain drain under later L2 matmuls).
            hsbs = {}
            for e in range(E):
                w1v = w1ts[e][:].rearrange("p (k h) -> p k h", k=KC1)
                hsb = hpool.tile([P, MC, P], f8, tag="h", name=f"hsb{e}")
                hsbs[e] = hsb
                for g in range(NG):
                    zp = psz.tile([P, NG, P], mybir.dt.float32, tag="zp")
                    for mc in range(NG):
                        m = g * NG + mc
                        for kc in range(KC1 // 2):
                            nc.tensor.matmul(
                                zp[:, mc],
                                lhsT=w1v[:, 2 * kc : 2 * kc + 2, m * P : (m + 1) * P],
                                rhs=ft[:, 2 * kc : 2 * kc + 2, :],
                                start=(kc == 0),
                                stop=(kc == KC1 // 2 - 1),
                                perf_mode=mybir.MatmulPerfMode.DoubleRow,
                            )
                    zb = zpool.tile([P, NG, P], mybir.dt.bfloat16, tag="zb")
                    nc.vector.tensor_tensor(
                        zb[:],
                        zp[:],
                        b1s[:, e, g * NG : (g + 1) * NG, None].to_broadcast([P, NG, P]),
                        mybir.AluOpType.add,
                    )
                    nc.scalar.activation(
                        hsb[:, g * NG : (g + 1) * NG],
                        zb[:],
                        act_fn,
                    )

            # Phase 2: layer-2 + loss accumulation. The +b2[e] term rides the
            # PE as a bf16 rank-1 matmul queued BEFORE the W2 data arrives;
            # the post-stream tail is one DoubleRow matmul + a DVE subtract
            # (PSUM may only feed ONE non-scalar DVE input) + a bf16-rate DVE
            # tensor_tensor_reduce.
            # per-expert row-sums land in one [P, E] tile; a single final
            # matmul folds partitions AND experts -> [1, E] (host sums 8
            # floats), keeping the per-expert DVE chain adds off the tail.
            redall = cpool.tile([P, E], f32)
            for e in range(E):
                hsb = hsbs[e]
                last = split_w2 and e == E - 1
                if last:
                    pairs = [(w2l[0], kc) for kc in range(6)] + [
                        (w2l[1], 0), (w2l[2], 0)]
                else:
                    pairs = [(w2ts[e], kc) for kc in range(MC // 2)]
                # (A half-split tail for the last expert saved ~0.5us but
                # showed a NaN flake in 1 of 3 hardware runs — keeping the
                # full-width chain, which was clean across every run.)
                halves = 1
                HW_ = D // halves
                pos = [
                    pso.tile([P, HW_], mybir.dt.float32, tag="po",
                             name=f"po{e}h{h}")
                    for h in range(halves)
                ]
                for i, (w2t, kc) in enumerate(pairs):
                    for h, po in enumerate(pos):
                        # start=True rides the FIRST DR matmul (whose natural
                        # wait is the W2 DMA sem) — putting it on the b2
                        # ones-matmul made each expert's group serialize
                        # behind the previous expert's DVE subtract.
                        nc.tensor.matmul(
                            po[:],
                            lhsT=hsb[:, 2 * i : 2 * i + 2, :],
                            rhs=w2t[:, kc, :, h * HW_ : (h + 1) * HW_],
                            start=(i == 0),
                            stop=(i == MC // 2 - 1),
                            perf_mode=mybir.MatmulPerfMode.DoubleRow,
                            skip_group_check=use_b2fold,
                        )
                        if i == 0 and use_b2fold:
                            nc.tensor.matmul(
                                po[:], lhsT=ones1[:],
                                rhs=b2s[:, e, h * HW_ : (h + 1) * HW_],
                                start=False, stop=False, skip_group_check=True,
                            )

                for h, po in enumerate(pos):
                    err = spool.tile([B, HW_], mybir.dt.bfloat16, tag="err",
                                     bufs=4)
                    tsrc = (tgt[:, h * HW_ : (h + 1) * HW_] if use_b2fold
                            else tgt2[:, e, h * HW_ : (h + 1) * HW_])
                    nc.vector.tensor_tensor(
                        err[:], po[:], tsrc, mybir.AluOpType.subtract
                    )
                    red = redall[:, e + h : e + h + 1]
                    sq = spool.tile([B, HW_], mybir.dt.bfloat16, tag="sq",
                                    bufs=4)
                    if loss_mode == "dve":
                        nc.vector.tensor_tensor(
                            sq[:], err[:], err[:], mybir.AluOpType.mult
                        )
                        nc.vector.tensor_reduce(
                            red, sq[:], mybir.AxisListType.X,
                            mybir.AluOpType.add
                        )
                    else:
                        nc.scalar.activation(
                            sq[:], err[:],
                            mybir.ActivationFunctionType.Square,
                            accum_out=red,
                        )

            # Cross-partition reduction on PE -> [1, E], one 32-byte output
            # descriptor; the host sums the 8 floats.
            pf = pso.tile([1, E], mybir.dt.float32, tag="po")
            nc.tensor.matmul(pf[:], lhsT=onesc[:], rhs=redall[:],
                             start=True, stop=True)
            osb = cpool.tile([1, E], mybir.dt.float32)
            nc.vector.tensor_copy(osb[:], pf[:])
            nc.sync.dma_start(loss[:], osb[:])

    nc.finalize()
    return nc


def get_nc(act="gelu"):
    global _NC
    if _NC is None:
        _NC = _build_nc(act)
    return _NC


def make_in_maps(features, target_features, W1, b1, W2, b2):
    features = np.asarray(features, np.float32)
    target_features = np.asarray(target_features, np.float32)
    W1 = np.asarray(W1, np.float32)
    b1 = np.asarray(b1, np.float32)
    W2 = np.asarray(W2, np.float32)
    b2 = np.asarray(b2, np.float32)

    # pack weights partition-major: wpack[a][e][p, kc*H + col] = W1[a,e,kc*128+p,col]
    #                              wpack[a][e][p, 8192 + kc*D + d] = W2[a,e,kc*128+p,d]
    w1p = np.ascontiguousarray(
        W1.reshape(E, E, KC1, P, H).transpose(0, 1, 3, 2, 4).reshape(E, E, P, W1_COLS)
    ).astype(F8)
    w2p = np.ascontiguousarray(
        W2.reshape(E, E, MC, P, D).transpose(0, 1, 3, 2, 4).reshape(E, E, P, W2_COLS)
    ).astype(F8)
    wpk = np.concatenate([w1p, w2p], axis=3)  # [A, E, P, 16384] fp8

    in_maps = []
    for a in range(E):
        fa = features[:, a]  # [B, D]
        ftp = np.ascontiguousarray(fa.T.reshape(KC1, P, B).transpose(1, 0, 2)).astype(F8)
        b1pa = np.ascontiguousarray(b1[a].reshape(E, MC, P).transpose(2, 0, 1))  # [P,E,MC]
        tgta = np.ascontiguousarray(target_features[:, a]).astype(BF16)  # [B, D]
        b2a = np.ascontiguousarray(b2[a][None]).astype(BF16)  # [1, E, D]
        t2a = np.ascontiguousarray(
            (target_features[:, a][:, None, :] - b2[a][None, :, :]).transpose(0, 1, 2)
        ).astype(BF16)  # [B, E, D]
        in_maps.append(
            {"wpack": wpk[a], "ftp": ftp, "b1p": b1pa,
             "tgtp": tgta, "b2pp": b2a, "t2p": t2a}
        )
    return in_maps


def kernel(features, target_features, W1, b1, W2, b2):
    from concourse.bass_utils import run_bass_kernel_spmd

    nc = get_nc()
    in_maps = make_in_maps(features, target_features, W1, b1, W2, b2)
    res = run_bass_kernel_spmd(nc, in_maps, list(range(E)))
    total = sum(float(np.asarray(r["loss"]).sum()) for r in res.results)
    return np.float32(total / (B * D * E))


# revision 54
# speedup vs baseline: 1.0431x; 1.0431x over previous
"""Trainium2 Bass kernel for nn_AveragedAdapter (dense_mlp).

Computes: loss = sum_{a,e} mean_{b,d} (gelu(f[:,a] @ W1[a,e] + b1[a,e]) @ W2[a,e]
                                        + b2[a,e] - target[:,a])^2 / E

Sharding: expert-parallel over the first expert axis `a` — core a computes the
full inner-e loop for its adapter row and returns per-partition partial sums of
squared errors; the host sums the 8x128 partials and applies the 1/(B*D*E)
scale.

The 512 MiB of weights dominate the roofline (each element used exactly once),
so weights (plus features and the hidden activations) are carried in fp8-e4m3.
Biases, targets and all accumulation stay >= bf16 (matmuls accumulate in fp32
PSUM).

Per-core program (a = core id):
  - W1[a],W2[a] packed host-side into one [E, 128, 16384] fp8 slab
    (partition-major; cols 0..8192 = W1 k-chunks, 8192..16384 = W2 k-chunk
    pairs). The sync HWDGE ring is FIFO, so slabs are issued in consumption
    order: all W1 slabs, then W2 slabs, with the LAST W2 slab split in three
    pieces (6/1/1 chunk-pairs) so its matmuls chase the stream — only ~128KB
    plus one DoubleRow matmul gate the loss tail after the final byte.
    Small inputs (features, b1, shared target, b2 rows) ride the scalar
    ring; the per-expert [B,E,D] target tiles of the original design are
    gone (~1MB less HBM traffic).
  - phase 1 (all experts): layer 1 computes hT (H on partitions) with W1
    chunk-pairs stationary via fp8 DoubleRow; bias add on DVE -> bf16;
    exact-erf Gelu on ACT -> fp8 h kept in SBUF for all 8 experts.
  - phase 2 (all experts): 8 fp8 DoubleRow matmuls accumulate h @ W2 into a
    PSUM bank; start=True rides the FIRST matmul (whose natural wait is the
    W2 DMA semaphore — carrying it on any other group member serializes each
    expert behind the previous expert's DVE subtract, ~1.3us/expert); b2[e]
    is folded in as a bf16 rank-1 ones-row matmul slotted second (start
    =False), so no [B,E,D] target prep is needed. Then err = psum - target
    on DVE (bf16), Square+row-accumulate on ACT into a column of one
    [128,8] accumulator. pso bufs=4 so the start-group zeroing never waits
    on a recent reader.
  - output: one final matmul folds partitions -> [1,8] (32B, single
    descriptor; a [128,1] output would emit 128 4-byte descriptors, each an
    HBM read-modify-write — measured ~8us). Host sums 8 floats per core.
"""

import sys

if "/opt/trn_rl_repo" not in sys.path:
    sys.path.insert(0, "/opt/trn_rl_repo")

import numpy as np
import ml_dtypes

B, E, D, M = 128, 8, 512, 4
H = M * D            # 2048
P = 128
KC1 = D // P         # 4  k-chunks in layer 1
MC = H // P          # 16 m-chunks of H / k-chunks in layer 2
NG = 4               # m-chunk groups (4 chunks -> one PSUM bank)
W1_COLS = KC1 * H    # 8192
W2_COLS = MC * D     # 8192
F8 = ml_dtypes.float8_e4m3
BF16 = ml_dtypes.bfloat16

_NC = None


def _build_nc(act="gelu", loss_mode="act", use_b2fold=True, split_w2=True):
    import concourse.tile as tile
    from concourse import bacc, mybir

    act_fn = {
        "gelu": mybir.ActivationFunctionType.Gelu,
        "identity": mybir.ActivationFunctionType.Identity,
    }[act]
    # Bacc (not Bass): its compile() pass legalizes sync waits for the trn2
    # ISA's one-wait-per-instruction limit.
    nc = bacc.Bacc(None)
    f8 = mybir.dt.float8e4
    f32 = mybir.dt.float32
    bf16 = mybir.dt.bfloat16

    wpack = nc.dram_tensor("wpack", [E, P, W1_COLS + W2_COLS], f8, kind="ExternalInput")
    ftp = nc.dram_tensor("ftp", [P, KC1, B], f8, kind="ExternalInput")
    b1p = nc.dram_tensor("b1p", [P, E, MC], f32, kind="ExternalInput")
    tgtp = nc.dram_tensor("tgtp", [B, D], bf16, kind="ExternalInput")   # target[:,a]
    b2pp = nc.dram_tensor("b2pp", [1, E, D], bf16, kind="ExternalInput")  # b2[a]
    # combined target+bias per inner expert (only DMA'd when use_b2fold=False)
    t2p = nc.dram_tensor("t2p", [P, E, D], bf16, kind="ExternalInput")
    # [1,E] single-partition output: a [128,1] output would emit 128
    # four-byte descriptors, each an HBM read-modify-write (sub-512B
    # transfers) — measured ~8us of post-kernel DMA time.
    loss = nc.dram_tensor("loss", [1, E], f32, kind="ExternalOutput")

    with tile.TileContext(nc) as tc:
        with (
            tc.tile_pool(name="w1pool", bufs=E) as w1pool,
            tc.tile_pool(name="w2pool", bufs=E + 2) as w2pool,
            tc.tile_pool(name="cpool", bufs=1) as cpool,
            tc.tile_pool(name="zpool", bufs=8) as zpool,
            tc.tile_pool(name="hpool", bufs=E) as hpool,
            tc.tile_pool(name="spool", bufs=2) as spool,
            tc.tile_pool(name="rpool", bufs=E) as rpool,
            tc.tile_pool(name="psz", bufs=4, space="PSUM") as psz,
            tc.tile_pool(name="pso", bufs=4, space="PSUM") as pso,
        ):
            # Small inputs on the scalar (ACT HWDGE) ring; weight slabs own the
            # sync ring end to end.
            ft = cpool.tile([P, KC1, B], f8)
            nc.scalar.dma_start(ft[:], ftp[:])
            b1s = cpool.tile([P, E, MC], f32)
            nc.scalar.dma_start(b1s[:], b1p[:])
            tgt = cpool.tile([B, D], bf16)
            nc.scalar.dma_start(tgt[:], tgtp[:])
            b2s = cpool.tile([1, E, D], bf16)
            nc.scalar.dma_start(b2s[:], b2pp[:])
            if not use_b2fold:
                tgt2 = cpool.tile([P, E, D], bf16)
                nc.scalar.dma_start(tgt2[:], t2p[:])
            ones1 = cpool.tile([1, P], bf16)
            nc.vector.memset(ones1[:], 1.0)
            # Advance the DVE vector clock past the b1s DMA with a one-element
            # read so the bias-add TTs only need their PE wait.
            dummy = cpool.tile([1, 2], f32)
            nc.vector.tensor_copy(dummy[:, 0:1], b1s[:1, 0, :1])
            # Trigger the ACT function-set table loads NOW (first use drives
            # the PSEUDO_LOAD_ACT_FUNC_SET) so their Q_XIV DMA packets drain
            # before the weight stream floods the rings — otherwise DMA
            # engine 0 carries them mid-stream and straggles ~2.5us behind
            # the other 15 engines, delaying the last W2 bytes by as much.
            dact = cpool.tile([1, 2], f32)
            nc.vector.memset(dact[:], 0.0)
            dact2 = cpool.tile([1, 2], f32)
            nc.scalar.activation(dact2[:], dact[:], act_fn)

            # Warm the PE HAM clock-gate (idle PE runs at 1.2 GHz; sustained
            # activity unlocks 2.4 GHz) while the first weight slab is in
            # flight. 8 matmuls ~= the first slab's flight time; real L1 work
            # continues the activity streak afterwards.
            onesc = cpool.tile([P, 1], f32)
            nc.vector.memset(onesc[:], 1.0)
            wsrc = cpool.tile([P, D], f8)
            nc.vector.memset(wsrc[:], 0.0)
            pwarm = psz.tile([P, D], mybir.dt.float32, tag="zp")
            NWARM = 15
            for i in range(NWARM):
                nc.tensor.matmul(
                    pwarm[:], lhsT=wsrc[:, :P], rhs=wsrc[:],
                    start=(i == 0), stop=(i == NWARM - 1),
                )

            # Weight slab delivery order == consumption order (FIFO ring).
            w1ts, w2ts = {}, {}

            # The dummy Square below writes two bytes INTO the first W1 tile:
            # a write-after-write dependency that holds the first weight DMA
            # (and the whole FIFO sync ring behind it) until the Square
            # retires — which is only after BOTH ACT function-set tables have
            # landed. Their Q_XIV packets otherwise interleave with the young
            # weight stream on one DMA engine, which then straggles ~2.5us
            # behind the other 15 for the rest of the kernel, delaying the
            # last W2 bytes (and the loss tail) by as much. ~2us later
            # stream start, but an even, earlier finish.
            w1ts[0] = w1pool.tile([P, W1_COLS], f8, tag="w1", name="w1t0")
            nc.scalar.activation(
                w1ts[0][0:1, 0:2], dact[:], mybir.ActivationFunctionType.Square
            )
            for e in range(E):
                if e > 0:
                    w1ts[e] = w1pool.tile(
                        [P, W1_COLS], f8, tag="w1", name=f"w1t{e}")
                nc.sync.dma_start(w1ts[e][:], wpack[e][:, :W1_COLS])
            w2view = {
                e: wpack[e][:, W1_COLS:].rearrange(
                    "p (k two d) -> p k two d", two=2, d=D
                )
                for e in range(E)
            }
            nfull = E - 1 if split_w2 else E
            for e in range(nfull):
                w2ts[e] = w2pool.tile([P, MC // 2, 2, D], f8, tag="w2", name=f"w2t{e}")
                nc.sync.dma_start(w2ts[e][:], w2view[e])
            if split_w2:
                # Last expert's W2 in three pieces (6/1/1 chunk-pairs) so only
                # the final 128KB gates the last matmul.
                w2l = [
                    w2pool.tile([P, 6, 2, D], f8, tag="w2", name="w2t7a"),
                    w2pool.tile([P, 1, 2, D], f8, tag="w2", name="w2t7b"),
                    w2pool.tile([P, 1, 2, D], f8, tag="w2", name="w2t7c"),
                ]
                nc.sync.dma_start(w2l[0][:], w2view[E - 1][:, 0:6])
                nc.sync.dma_start(w2l[1][:], w2view[E - 1][:, 6:7])
                nc.sync.dma_start(w2l[2][:], w2view[E - 1][:, 7:8])

            # Phase 1: layer-1 + gelu for ALL experts (PE executes its queue in
            # program order; keeping layer-2 work out of this stretch lets the
            # last expert's bias/gelu chain drain under later L2 matmuls).
            hsbs = {}
            for e in range(E):
                w1v = w1ts[e][:].rearrange("p (k h) -> p k h", k=KC1)
                hsb = hpool.tile([P, MC, P], f8, tag="h", name=f"hsb{e}")
                hsbs[e] = hsb
                for g in range(NG):
                    zp = psz.tile([P, NG, P], mybir.dt.float32, tag="zp")
                    for mc in range(NG):
                        m = g * NG + mc
                        for kc in range(KC1 // 2):
                            nc.tensor.matmul(
                                zp[:, mc],
                                lhsT=w1v[:, 2 * kc : 2 * kc + 2, m * P : (m + 1) * P],
                                rhs=ft[:, 2 * kc : 2 * kc + 2, :],
                                start=(kc == 0),
                                stop=(kc == KC1 // 2 - 1),
                                perf_mode=mybir.MatmulPerfMode.DoubleRow,
                            )
                    zb = zpool.tile([P, NG, P], mybir.dt.bfloat16, tag="zb")
                    nc.vector.tensor_tensor(
                        zb[:],
                        zp[:],
                        b1s[:, e, g * NG : (g + 1) * NG, None].to_broadcast([P, NG, P]),
                        mybir.AluOpType.add,
                    )
                    nc.scalar.activation(
                        hsb[:, g * NG : (g + 1) * NG],
                        zb[:],
                        act_fn,
                    )

            # Phase 2: layer-2 + loss accumulation. The +b2[e] term rides the
            # PE as a bf16 rank-1 matmul queued BEFORE the W2 data arrives;
            # the post-stream tail is one DoubleRow matmul + a DVE subtract
            # (PSUM may only feed ONE non-scalar DVE input) + a bf16-rate DVE
            # tensor_tensor_reduce.
            # per-expert row-sums land in one [P, E] tile; a single final
            # matmul folds partitions AND experts -> [1, E] (host sums 8
            # floats), keeping the per-expert DVE chain adds off the tail.
            redall = cpool.tile([P, E], f32)
            for e in range(E):
                hsb = hsbs[e]
                last = split_w2 and e == E - 1
                if last:
                    pairs = [(w2l[0], kc) for kc in range(6)] + [
                        (w2l[1], 0), (w2l[2], 0)]
                else:
                    pairs = [(w2ts[e], kc) for kc in range(MC // 2)]
                # (A half-split tail for the last expert saved ~0.5us but
                # showed a NaN flake in 1 of 3 hardware runs — keeping the
                # full-width chain, which was clean across every run.)
                halves = 1
                HW_ = D // halves
                pos = [
                    pso.tile([P, HW_], mybir.dt.float32, tag="po",
                             name=f"po{e}h{h}")
                    for h in range(halves)
                ]
                for i, (w2t, kc) in enumerate(pairs):
                    for h, po in enumerate(pos):
                        # start=True rides the FIRST DR matmul (whose natural
                        # wait is the W2 DMA sem) — putting it on the b2
                        # ones-matmul made each expert's group serialize
                        # behind the previous expert's DVE subtract.
                        nc.tensor.matmul(
                            po[:],
                            lhsT=hsb[:, 2 * i : 2 * i + 2, :],
                            rhs=w2t[:, kc, :, h * HW_ : (h + 1) * HW_],
                            start=(i == 0),
                            stop=(i == MC // 2 - 1),
                            perf_mode=mybir.MatmulPerfMode.DoubleRow,
                            skip_group_check=use_b2fold,
                        )
                        if i == 0 and use_b2fold:
                            nc.tensor.matmul(
                                po[:], lhsT=ones1[:],
                                rhs=b2s[:, e, h * HW_ : (h + 1) * HW_],
                                start=False, stop=False, skip_group_check=True,
                            )

                for h, po in enumerate(pos):
                    err = spool.tile([B, HW_], mybir.dt.bfloat16, tag="err",
                                     bufs=4)
                    tsrc = (tgt[:, h * HW_ : (h + 1) * HW_] if use_b2fold
                            else tgt2[:, e, h * HW_ : (h + 1) * HW_])
                    nc.vector.tensor_tensor(
                        err[:], po[:], tsrc, mybir.AluOpType.subtract
                    )
                    red = redall[:, e + h : e + h + 1]
                    sq = spool.tile([B, HW_], mybir.dt.bfloat16, tag="sq",
                                    bufs=4)
                    if loss_mode == "dve":
                        nc.vector.tensor_tensor(
                            sq[:], err[:], err[:], mybir.AluOpType.mult
                        )
                        nc.vector.tensor_reduce(
                            red, sq[:], mybir.AxisListType.X,
                            mybir.AluOpType.add
                        )
                    else:
                        nc.scalar.activation(
                            sq[:], err[:],
                            mybir.ActivationFunctionType.Square,
                            accum_out=red,
                        )

            # Cross-partition reduction on PE -> [1, E], one 32-byte output
            # descriptor; the host sums the 8 floats.
            pf = pso.tile([1, E], mybir.dt.float32, tag="po")
            nc.tensor.matmul(pf[:], lhsT=onesc[:], rhs=redall[:],
                             start=True, stop=True)
            osb = cpool.tile([1, E], mybir.dt.float32)
            nc.vector.tensor_copy(osb[:], pf[:])
            nc.sync.dma_start(loss[:], osb[:])

    nc.finalize()
    return nc


def get_nc(act="gelu"):
    global _NC
    if _NC is None:
        _NC = _build_nc(act)
    return _NC


def make_in_maps(features, target_features, W1, b1, W2, b2):
    features = np.asarray(features, np.float32)
    target_features = np.asarray(target_features, np.float32)
    W1 = np.asarray(W1, np.float32)
    b1 = np.asarray(b1, np.float32)
    W2 = np.asarray(W2, np.float32)
    b2 = np.asarray(b2, np.float32)

    # pack weights partition-major: wpack[a][e][p, kc*H + col] = W1[a,e,kc*128+p,col]
    #                              wpack[a][e][p, 8192 + kc*D + d] = W2[a,e,kc*128+p,d]
    w1p = np.ascontiguousarray(
        W1.reshape(E, E, KC1, P, H).transpose(0, 1, 3, 2, 4).reshape(E, E, P, W1_COLS)
    ).astype(F8)
    w2p = np.ascontiguousarray(
        W2.reshape(E, E, MC, P, D).transpose(0, 1, 3, 2, 4).reshape(E, E, P, W2_COLS)
    ).astype(F8)
    wpk = np.concatenate([w1p, w2p], axis=3)  # [A, E, P, 16384] fp8

    in_maps = []
    for a in range(E):
        fa = features[:, a]  # [B, D]
        ftp = np.ascontiguousarray(fa.T.reshape(KC1, P, B).transpose(1, 0, 2)).astype(F8)
        b1pa = np.ascontiguousarray(b1[a].reshape(E, MC, P).transpose(2, 0, 1))  # [P,E,MC]
        tgta = np.ascontiguousarray(target_features[:, a]).astype(BF16)  # [B, D]
        b2a = np.ascontiguousarray(b2[a][None]).astype(BF16)  # [1, E, D]
        t2a = np.ascontiguousarray(
            (target_features[:, a][:, None, :] - b2[a][None, :, :]).transpose(0, 1, 2)
        ).astype(BF16)  # [B, E, D]
        in_maps.append(
            {"wpack": wpk[a], "ftp": ftp, "b1p": b1pa,
             "tgtp": tgta, "b2pp": b2a, "t2p": t2a}
        )
    return in_maps


def kernel(features, target_features, W1, b1, W2, b2):
    from concourse.bass_utils import run_bass_kernel_spmd

    nc = get_nc()
    in_maps = make_in_maps(features, target_features, W1, b1, W2, b2)
    res = run_bass_kernel_spmd(nc, in_maps, list(range(E)))
    total = sum(float(np.asarray(r["loss"]).sum()) for r in res.results)
    return np.float32(total / (B * D * E))


# revision 60
# speedup vs baseline: 1.1536x; 1.1059x over previous
"""Trainium2 Bass kernel for nn_AveragedAdapter (dense_mlp).

Computes: loss = sum_{a,e} mean_{b,d} (gelu(f[:,a] @ W1[a,e] + b1[a,e]) @ W2[a,e]
                                        + b2[a,e] - target[:,a])^2 / E

Sharding: expert-parallel over the first expert axis `a` — core a computes the
full inner-e loop for its adapter row and returns per-partition partial sums of
squared errors; the host sums the 8x128 partials and applies the 1/(B*D*E)
scale.

The 512 MiB of weights dominate the roofline (each element used exactly once),
so weights (plus features and the hidden activations) are carried in fp8-e4m3.
Biases, targets and all accumulation stay >= bf16 (matmuls accumulate in fp32
PSUM).

Per-core program (a = core id):
  - W1[a],W2[a] packed host-side into one [E, 128, 16384] fp8 slab
    (partition-major; cols 0..8192 = W1 k-chunks, 8192..16384 = W2 k-chunk
    pairs). The sync HWDGE ring is FIFO, so slabs are issued in consumption
    order: all W1 slabs, then W2 slabs, with the LAST W2 slab split in three
    pieces (6/1/1 chunk-pairs) so its matmuls chase the stream — only ~128KB
    plus one DoubleRow matmul gate the loss tail after the final byte.
    Small inputs (features, b1, shared target, b2 rows) ride the scalar
    ring; the per-expert [B,E,D] target tiles of the original design are
    gone (~1MB less HBM traffic).
  - phase 1 (all experts): layer 1 computes hT (H on partitions) with W1
    chunk-pairs stationary via fp8 DoubleRow; bias add on DVE -> bf16;
    exact-erf Gelu on ACT -> fp8 h kept in SBUF for all 8 experts.
  - phase 2 (all experts): 8 fp8 DoubleRow matmuls accumulate h @ W2 into a
    PSUM bank; start=True rides the FIRST matmul (whose natural wait is the
    W2 DMA semaphore — carrying it on any other group member serializes each
    expert behind the previous expert's DVE subtract, ~1.3us/expert); b2[e]
    is folded in as a bf16 rank-1 ones-row matmul slotted second (start
    =False), so no [B,E,D] target prep is needed. Then err = psum - target
    on DVE (bf16), Square+row-accumulate on ACT into a column of one
    [128,8] accumulator. pso bufs=4 so the start-group zeroing never waits
    on a recent reader.
  - output: one final matmul folds partitions -> [1,8] (32B, single
    descriptor; a [128,1] output would emit 128 4-byte descriptors, each an
    HBM read-modify-write — measured ~8us). Host sums 8 floats per core.
"""

import sys

if "/opt/trn_rl_repo" not in sys.path:
    sys.path.insert(0, "/opt/trn_rl_repo")

import numpy as np
import ml_dtypes

B, E, D, M = 128, 8, 512, 4
H = M * D            # 2048
P = 128
KC1 = D // P         # 4  k-chunks in layer 1
MC = H // P          # 16 m-chunks of H / k-chunks in layer 2
NG = 4               # m-chunk groups (4 chunks -> one PSUM bank)
W1_COLS = KC1 * H    # 8192
W2_COLS = MC * D     # 8192
F8 = ml_dtypes.float8_e4m3
BF16 = ml_dtypes.bfloat16

_NC = None


def _build_nc(act="gelu", loss_mode="act", use_b2fold=True, split_w2=True):
    import concourse.tile as tile
    from concourse import bacc, mybir

    act_fn = {
        "gelu": mybir.ActivationFunctionType.Gelu,
        "identity": mybir.ActivationFunctionType.Identity,
    }[act]
    # Bacc (not Bass): its compile() pass legalizes sync waits for the trn2
    # ISA's one-wait-per-instruction limit.
    nc = bacc.Bacc(None)
    f8 = mybir.dt.float8e4
    f32 = mybir.dt.float32
    bf16 = mybir.dt.bfloat16

    wpack = nc.dram_tensor("wpack", [E, P, W1_COLS + W2_COLS], f8, kind="ExternalInput")
    ftp = nc.dram_tensor("ftp", [P, KC1, B], f8, kind="ExternalInput")
    b1p = nc.dram_tensor("b1p", [P, E, MC], f32, kind="ExternalInput")
    tgtp = nc.dram_tensor("tgtp", [B, D], bf16, kind="ExternalInput")   # -target[:,a]
    b2pp = nc.dram_tensor("b2pp", [1, E, D], bf16, kind="ExternalInput")  # b2[a]
    idn = nc.dram_tensor("idn", [P, P], bf16, kind="ExternalInput")     # identity
    # combined target+bias per inner expert (only DMA'd when use_b2fold=False)
    t2p = nc.dram_tensor("t2p", [P, E, D], bf16, kind="ExternalInput")
    # [1,E] single-partition output: a [128,1] output would emit 128
    # four-byte descriptors, each an HBM read-modify-write (sub-512B
    # transfers) — measured ~8us of post-kernel DMA time.
    loss = nc.dram_tensor("loss", [1, E], f32, kind="ExternalOutput")

    with tile.TileContext(nc) as tc:
        with (
            tc.tile_pool(name="w1pool", bufs=E) as w1pool,
            tc.tile_pool(name="w2pool", bufs=E + 2) as w2pool,
            tc.tile_pool(name="cpool", bufs=1) as cpool,
            tc.tile_pool(name="zpool", bufs=8) as zpool,
            tc.tile_pool(name="hpool", bufs=E) as hpool,
            tc.tile_pool(name="spool", bufs=2) as spool,
            tc.tile_pool(name="rpool", bufs=E) as rpool,
            tc.tile_pool(name="psz", bufs=4, space="PSUM") as psz,
            tc.tile_pool(name="pso", bufs=4, space="PSUM") as pso,
        ):
            # Small inputs on the scalar (ACT HWDGE) ring; weight slabs own the
            # sync ring end to end.
            ft = cpool.tile([P, KC1, B], f8)
            nc.scalar.dma_start(ft[:], ftp[:])
            b1s = cpool.tile([P, E, MC], f32)
            nc.scalar.dma_start(b1s[:], b1p[:])
            tgt = cpool.tile([B, D], bf16)
            nc.scalar.dma_start(tgt[:], tgtp[:])
            b2s = cpool.tile([1, E, D], bf16)
            nc.scalar.dma_start(b2s[:], b2pp[:])
            ident = cpool.tile([P, P], bf16)
            nc.scalar.dma_start(ident[:], idn[:])
            if not use_b2fold:
                tgt2 = cpool.tile([P, E, D], bf16)
                nc.scalar.dma_start(tgt2[:], t2p[:])
            ones1 = cpool.tile([1, P], bf16)
            nc.vector.memset(ones1[:], 1.0)
            # Advance the DVE vector clock past the b1s DMA with a one-element
            # read so the bias-add TTs only need their PE wait.
            dummy = cpool.tile([1, 2], f32)
            nc.vector.tensor_copy(dummy[:, 0:1], b1s[:1, 0, :1])
            # Trigger the ACT function-set table loads NOW (first use drives
            # the PSEUDO_LOAD_ACT_FUNC_SET) so their Q_XIV DMA packets drain
            # before the weight stream floods the rings — otherwise DMA
            # engine 0 carries them mid-stream and straggles ~2.5us behind
            # the other 15 engines, delaying the last W2 bytes by as much.
            dact = cpool.tile([1, 2], f32)
            nc.vector.memset(dact[:], 0.0)
            dact2 = cpool.tile([1, 2], f32)
            nc.scalar.activation(dact2[:], dact[:], act_fn)
            nc.scalar.activation(
                dact2[:], dact[:], mybir.ActivationFunctionType.Square
            )

            # Warm the PE HAM clock-gate (idle PE runs at 1.2 GHz; sustained
            # activity unlocks 2.4 GHz) while the first weight slab is in
            # flight. 8 matmuls ~= the first slab's flight time; real L1 work
            # continues the activity streak afterwards.
            onesc = cpool.tile([P, 1], f32)
            nc.vector.memset(onesc[:], 1.0)
            wsrc = cpool.tile([P, D], f8)
            nc.vector.memset(wsrc[:], 0.0)
            pwarm = psz.tile([P, D], mybir.dt.float32, tag="zp")
            NWARM = 15
            for i in range(NWARM):
                nc.tensor.matmul(
                    pwarm[:], lhsT=wsrc[:, :P], rhs=wsrc[:],
                    start=(i == 0), stop=(i == NWARM - 1),
                )

            # Weight slab delivery order == consumption order (FIFO ring).
            w1ts, w2ts = {}, {}

            for e in range(E):
                w1ts[e] = w1pool.tile([P, W1_COLS], f8, tag="w1", name=f"w1t{e}")
                nc.sync.dma_start(w1ts[e][:], wpack[e][:, :W1_COLS])
            w2view = {
                e: wpack[e][:, W1_COLS:].rearrange(
                    "p (k two d) -> p k two d", two=2, d=D
                )
                for e in range(E)
            }
            nfull = E - 1 if split_w2 else E
            for e in range(nfull):
                w2ts[e] = w2pool.tile([P, MC // 2, 2, D], f8, tag="w2", name=f"w2t{e}")
                nc.sync.dma_start(w2ts[e][:], w2view[e])
            if split_w2:
                # Last expert's W2 in three pieces (6/1/1 chunk-pairs) so only
                # the final 128KB gates the last matmul.
                w2l = [
                    w2pool.tile([P, 6, 2, D], f8, tag="w2", name="w2t7a"),
                    w2pool.tile([P, 1, 2, D], f8, tag="w2", name="w2t7b"),
                    w2pool.tile([P, 1, 2, D], f8, tag="w2", name="w2t7c"),
                ]
                nc.sync.dma_start(w2l[0][:], w2view[E - 1][:, 0:6])
                nc.sync.dma_start(w2l[1][:], w2view[E - 1][:, 6:7])
                nc.sync.dma_start(w2l[2][:], w2view[E - 1][:, 7:8])

            # Phase 1: layer-1 + gelu for ALL experts (PE executes its queue in
            # program order; keeping layer-2 work out of this stretch lets the
            # last expert's bias/gelu chain drain under later L2 matmuls).
            hsbs = {}
            for e in range(E):
                w1v = w1ts[e][:].rearrange("p (k h) -> p k h", k=KC1)
                hsb = hpool.tile([P, MC, P], f8, tag="h", name=f"hsb{e}")
                hsbs[e] = hsb
                for g in range(NG):
                    zp = psz.tile([P, NG, P], mybir.dt.float32, tag="zp")
                    for mc in range(NG):
                        m = g * NG + mc
                        for kc in range(KC1 // 2):
                            nc.tensor.matmul(
                                zp[:, mc],
                                lhsT=w1v[:, 2 * kc : 2 * kc + 2, m * P : (m + 1) * P],
                                rhs=ft[:, 2 * kc : 2 * kc + 2, :],
                                start=(kc == 0),
                                stop=(kc == KC1 // 2 - 1),
                                perf_mode=mybir.MatmulPerfMode.DoubleRow,
                            )
                    zb = zpool.tile([P, NG, P], mybir.dt.bfloat16, tag="zb")
                    nc.vector.tensor_tensor(
                        zb[:],
                        zp[:],
                        b1s[:, e, g * NG : (g + 1) * NG, None].to_broadcast([P, NG, P]),
                        mybir.AluOpType.add,
                    )
                    nc.scalar.activation(
                        hsb[:, g * NG : (g + 1) * NG],
                        zb[:],
                        act_fn,
                    )

            # Phase 2: layer-2 + loss accumulation. The +b2[e] term rides the
            # PE as a bf16 rank-1 matmul queued BEFORE the W2 data arrives;
            # the post-stream tail is one DoubleRow matmul + a DVE subtract
            # (PSUM may only feed ONE non-scalar DVE input) + a bf16-rate DVE
            # tensor_tensor_reduce.
            # per-expert row-sums land in one [P, E] tile; a single final
            # matmul folds partitions AND experts -> [1, E] (host sums 8
            # floats), keeping the per-expert DVE chain adds off the tail.
            redall = cpool.tile([P, E], f32)
            fold7 = use_b2fold and loss_mode == "act"
            for e in range(E):
                hsb = hsbs[e]
                last = split_w2 and e == E - 1
                if last:
                    pairs = [(w2l[0], kc) for kc in range(6)] + [
                        (w2l[1], 0), (w2l[2], 0)]
                else:
                    pairs = [(w2ts[e], kc) for kc in range(MC // 2)]
                # (A half-split tail for the last expert saved ~0.5us but
                # showed a NaN flake in 1 of 3 hardware runs — keeping the
                # full-width chain, which was clean across every run.)
                halves = 1
                HW_ = D // halves
                pos = [
                    pso.tile([P, HW_], mybir.dt.float32, tag="po",
                             name=f"po{e}h{h}")
                    for h in range(halves)
                ]
                for i, (w2t, kc) in enumerate(pairs):
                    for h, po in enumerate(pos):
                        # start=True rides the FIRST DR matmul (whose natural
                        # wait is the W2 DMA sem) — putting it on the b2
                        # ones-matmul made each expert's group serialize
                        # behind the previous expert's DVE subtract.
                        nc.tensor.matmul(
                            po[:],
                            lhsT=hsb[:, 2 * i : 2 * i + 2, :],
                            rhs=w2t[:, kc, :, h * HW_ : (h + 1) * HW_],
                            start=(i == 0),
                            stop=(i == MC // 2 - 1),
                            perf_mode=mybir.MatmulPerfMode.DoubleRow,
                            skip_group_check=use_b2fold,
                        )
                        if i == 0 and use_b2fold:
                            nc.tensor.matmul(
                                po[:], lhsT=ones1[:],
                                rhs=b2s[:, e, h * HW_ : (h + 1) * HW_],
                                start=False, stop=False, skip_group_check=True,
                            )
                            if fold7 and last:
                                # fold -target into the LAST expert's PSUM
                                # with an identity matmul queued mid-group
                                # (it runs long before the final W2 piece
                                # lands): ACT Square then reads the PSUM
                                # residual directly and the DVE subtract
                                # drops off the post-stream critical tail.
                                nc.tensor.matmul(
                                    po[:], lhsT=ident[:],
                                    rhs=tgt[:, h * HW_ : (h + 1) * HW_],
                                    start=False, stop=False,
                                    skip_group_check=True,
                                )

                for h, po in enumerate(pos):
                    red = redall[:, e + h : e + h + 1]
                    sq = spool.tile([B, HW_], mybir.dt.bfloat16, tag="sq",
                                    bufs=4)
                    if fold7 and last:
                        nc.scalar.activation(
                            sq[:], po[:],
                            mybir.ActivationFunctionType.Square,
                            accum_out=red,
                        )
                        continue
                    err = spool.tile([B, HW_], mybir.dt.bfloat16, tag="err",
                                     bufs=4)
                    if use_b2fold:
                        # tgt holds -target, so err = po + tgt
                        nc.vector.tensor_tensor(
                            err[:], po[:], tgt[:, h * HW_ : (h + 1) * HW_],
                            mybir.AluOpType.add
                        )
                    else:
                        nc.vector.tensor_tensor(
                            err[:], po[:], tgt2[:, e, h * HW_ : (h + 1) * HW_],
                            mybir.AluOpType.subtract
                        )
                    if loss_mode == "dve":
                        nc.vector.tensor_tensor(
                            sq[:], err[:], err[:], mybir.AluOpType.mult
                        )
                        nc.vector.tensor_reduce(
                            red, sq[:], mybir.AxisListType.X,
                            mybir.AluOpType.add
                        )
                    else:
                        nc.scalar.activation(
                            sq[:], err[:],
                            mybir.ActivationFunctionType.Square,
                            accum_out=red,
                        )

            # Cross-partition reduction on PE -> [1, E], one 32-byte output
            # descriptor; the host sums the 8 floats.
            pf = pso.tile([1, E], mybir.dt.float32, tag="po")
            nc.tensor.matmul(pf[:], lhsT=onesc[:], rhs=redall[:],
                             start=True, stop=True)
            osb = cpool.tile([1, E], mybir.dt.float32)
            nc.vector.tensor_copy(osb[:], pf[:])
            nc.sync.dma_start(loss[:], osb[:])

    nc.finalize()
    return nc


def get_nc(act="gelu"):
    global _NC
    if _NC is None:
        _NC = _build_nc(act)
    return _NC


def make_in_maps(features, target_features, W1, b1, W2, b2):
    features = np.asarray(features, np.float32)
    target_features = np.asarray(target_features, np.float32)
    W1 = np.asarray(W1, np.float32)
    b1 = np.asarray(b1, np.float32)
    W2 = np.asarray(W2, np.float32)
    b2 = np.asarray(b2, np.float32)

    # pack weights partition-major: wpack[a][e][p, kc*H + col] = W1[a,e,kc*128+p,col]
    #                              wpack[a][e][p, 8192 + kc*D + d] = W2[a,e,kc*128+p,d]
    w1p = np.ascontiguousarray(
        W1.reshape(E, E, KC1, P, H).transpose(0, 1, 3, 2, 4).reshape(E, E, P, W1_COLS)
    ).astype(F8)
    w2p = np.ascontiguousarray(
        W2.reshape(E, E, MC, P, D).transpose(0, 1, 3, 2, 4).reshape(E, E, P, W2_COLS)
    ).astype(F8)
    wpk = np.concatenate([w1p, w2p], axis=3)  # [A, E, P, 16384] fp8

    ident = np.eye(P, dtype=BF16)
    in_maps = []
    for a in range(E):
        fa = features[:, a]  # [B, D]
        ftp = np.ascontiguousarray(fa.T.reshape(KC1, P, B).transpose(1, 0, 2)).astype(F8)
        b1pa = np.ascontiguousarray(b1[a].reshape(E, MC, P).transpose(2, 0, 1))  # [P,E,MC]
        tgta = np.ascontiguousarray(-target_features[:, a]).astype(BF16)  # [B, D], negated
        b2a = np.ascontiguousarray(b2[a][None]).astype(BF16)  # [1, E, D]
        t2a = np.ascontiguousarray(
            (target_features[:, a][:, None, :] - b2[a][None, :, :]).transpose(0, 1, 2)
        ).astype(BF16)  # [B, E, D]
        in_maps.append(
            {"wpack": wpk[a], "ftp": ftp, "b1p": b1pa,
             "tgtp": tgta, "b2pp": b2a, "t2p": t2a, "idn": ident}
        )
    return in_maps


def kernel(features, target_features, W1, b1, W2, b2):
    from concourse.bass_utils import run_bass_kernel_spmd

    nc = get_nc()
    in_maps = make_in_maps(features, target_features, W1, b1, W2, b2)
    res = run_bass_kernel_spmd(nc, in_maps, list(range(E)))
    total = sum(float(np.asarray(r["loss"]).sum()) for r in res.results)
    return np.float32(total / (B * D * E))
